# revision 11
# baseline (speedup 1.0000x reference)
"""Multi-head causal attention (B=2, S=2048, E=1024, H=16, D=64) on 8 trn2 cores.

Sharding: 32 (batch, head) pairs -> core c handles batch c//4, heads
4*(c%4) .. 4*(c%4)+4.  Each core computes QKV projections for its 4 heads,
causal flash-style attention, and a partial output projection (its heads'
rows of Wproj).  Host pre-transposes x (x^T is what every on-core matmul
wants), sums the 4 partials per batch, and adds bproj.

On-core layouts (partition dim first):
  xT   [128, 8, 2048]      x^T, e on partitions (8 per-tile DMAs)
  QT   [128, 2, 2048]      q^T pair-packed: rows 0-63 head even, 64-127 head odd
  KTz  [128, 4, 2048]      per-head k^T, zero-padded to K=128: head h occupies
                           rows 64*(h%2)..64*(h%2)+64, the other 64 rows are 0
  V    [128, 16, 4, 72]    v blocks [k, s-tile, head, d], col 64 = ones (denom)
  O    [128, 4, 2048]      per-head normalized attention output^T in rows 0-63,
                           rows 64-127 zero (K=128 projection)

Every matmul contracts over the full 128 partitions (zero-padded where the
logical contraction is 64) so the PE activity monitor keeps the array at
2.4 GHz.  Scores for a head pair share one rhs (the pair-packed Q^T tile);
one ScalarE exp covers both heads' [128, 2x512] block.  Softmax denominators
come from V's ones column; causal masking is a 0/1 broadcast-multiply on exp
tiles of diagonal blocks.  All matmuls run fp32r (TF32-like, full PE rate).
partition_broadcast on HW only reads partition 0, so denominators are moved
to partition 0 via a small SBUF->SBUF DMA before reciprocal+broadcast.
"""

import numpy as np

import concourse.bacc as bacc
import concourse.mybir as mybir
import concourse.tile as tile
from concourse import bass_utils

B, S, E, H, D = 2, 2048, 1024, 16, 64
NCORES = 8
HPC = 4            # heads per core
P = 128
QS = 512           # q slice width
NQS = S // QS      # 4
NKB = S // P       # 16 k blocks
FP32 = mybir.dt.float32
FP32R = mybir.dt.float32r
EXP = mybir.ActivationFunctionType.Exp

_CACHE = {}


def _build_program():
    if "nc" in _CACHE:
        return _CACHE["nc"]

    nc = bacc.Bacc("TRN2", target_bir_lowering=False, debug=False)
    xt_d = nc.dram_tensor("xt", (E, S), FP32, kind="ExternalInput").ap()
    wq_d = nc.dram_tensor("wq", (E, HPC * D), FP32, kind="ExternalInput").ap()
    wk_d = nc.dram_tensor("wk", (E, HPC * D), FP32, kind="ExternalInput").ap()
    wv_d = nc.dram_tensor("wv", (E, HPC * D), FP32, kind="ExternalInput").ap()
    wp_d = nc.dram_tensor("wp", (HPC * D, E), FP32, kind="ExternalInput").ap()
    msk_d = nc.dram_tensor("msk", (P, 4, QS), FP32, kind="ExternalInput").ap()
    one_d = nc.dram_tensor("one", (P, NKB * HPC), FP32, kind="ExternalInput").ap()
    zero_d = nc.dram_tensor("zero", (D, HPC * S), FP32, kind="ExternalInput").ap()
    y_d = nc.dram_tensor("y", (S, E), FP32, kind="ExternalOutput").ap()

    with tile.TileContext(nc) as tc:
        _kernel(tc, nc, xt_d, wq_d, wk_d, wv_d, wp_d, msk_d, one_d, zero_d, y_d)
    nc.compile()
    _CACHE["nc"] = nc
    return nc


def _kernel(tc, nc, xt_d, wq_d, wk_d, wv_d, wp_d, msk_d, one_d, zero_d, y_d):
    ET = E // P  # 8 e-tiles
    zrow = zero_d.bitcast(FP32R)
    with tc.tile_pool(name="persist", bufs=1) as persist:
        QT = persist.tile([P, 2, S], FP32R)
        KTz = persist.tile([P, HPC, S], FP32R)
        V = persist.tile([P, NKB, HPC, 72], FP32R)
        msk = persist.tile([P, 4, QS], FP32R)

        nc.sync.dma_start(msk, msk_d.bitcast(FP32R))
        nc.sync.dma_start(
            V[:, :, :, 64],
            one_d.rearrange("p (a b) -> p a b", a=NKB).bitcast(FP32R),
        )
        # zero the dead half of each head's K^T (K=128 contraction padding)
        for h in range(HPC):
            r = slice(0, D) if h % 2 else slice(D, P)
            nc.sync.dma_start(
                KTz[r, h, :], zrow[:, 0:S].rearrange("p s -> p s")
            )

        # ---- phase A: load x^T + QKV projections ----
        with (
            tc.tile_pool(name="qkvw", bufs=1) as qkvw,
            tc.tile_pool(name="qkvps", bufs=3, space="PSUM") as qkvps,
        ):
            wq_sb = qkvw.tile([P, ET, HPC * D], FP32R)
            wk_sb = qkvw.tile([P, ET, HPC * D], FP32R)
            wv_sb = qkvw.tile([P, ET, HPC * D], FP32R)
            for w_sb, w_d in ((wq_sb, wq_d), (wk_sb, wk_d), (wv_sb, wv_d)):
                nc.sync.dma_start(
                    w_sb, w_d.rearrange("(et p) m -> p et m", p=P).bitcast(FP32R)
                )
            xT = qkvw.tile([P, ET, S], FP32R)
            xt_r = xt_d.rearrange("(et p) s -> p et s", p=P).bitcast(FP32R)
            for et in range(ET):
                nc.sync.dma_start(xT[:, et, :], xt_r[:, et, :])

            # Q^T / K^T: head pairs packed on partitions
            for pr in range(2):
                he, ho = 2 * pr, 2 * pr + 1
                for ss in range(NQS):
                    ssl = slice(ss * QS, (ss + 1) * QS)
                    for w_sb, is_q in ((wq_sb, True), (wk_sb, False)):
                        ps = qkvps.tile([P, QS], FP32, tag="qkps")
                        for et in range(ET):
                            nc.tensor.matmul(
                                ps,
                                w_sb[:, et, 2 * D * pr:2 * D * (pr + 1)],
                                xT[:, et, ssl],
                                start=(et == 0), stop=(et == ET - 1),
                            )
                        if is_q:
                            nc.vector.tensor_copy(QT[:, pr, ssl], ps)
                        else:
                            nc.vector.tensor_copy(KTz[0:D, he, ssl], ps[0:D, :])
                            nc.vector.tensor_copy(KTz[D:P, ho, ssl], ps[D:P, :])

            # V: [s, head*d] via xT as stationary
            for st in range(NKB):
                ps = qkvps.tile([P, HPC * D], FP32, tag="vps")
                for et in range(ET):
                    nc.tensor.matmul(
                        ps,
                        xT[:, et, st * P:(st + 1) * P],
                        wv_sb[:, et, :],
                        start=(et == 0), stop=(et == ET - 1),
                    )
                for h in range(HPC):
                    nc.vector.tensor_copy(V[:, st, h, 0:D], ps[:, D * h:D * (h + 1)])

        # ---- phase B: attention, one head-pair per unit ----
        with tc.tile_pool(name="attn", bufs=1) as attn_pool:
            O = attn_pool.tile([P, HPC, S], FP32R)
            # Per-parity wp views so the K=128 projection pairs head data in
            # O rows 0-63 with that head's wp rows on partitions 0-63; rows
            # 64-127 multiply O's zero rows (values only need to be finite).
            wp_ev = attn_pool.tile([P, 2, E], FP32R)
            wp_od = attn_pool.tile([P, 2, E], FP32R)
            wpr = wp_d.bitcast(FP32R)
            for pr in range(2):
                nc.sync.dma_start(wp_ev[0:D, pr, :], wpr[P * pr:P * pr + D, :])
                nc.sync.dma_start(wp_ev[D:P, pr, :], wpr[P * pr + D:P * pr + P, :])
                nc.sync.dma_start(wp_od[0:D, pr, :], wpr[P * pr + D:P * pr + P, :])
                nc.sync.dma_start(wp_od[D:P, pr, :], wpr[P * pr:P * pr + D, :])
            # zero rows 64-127 of O (K=128 projection padding)
            nc.sync.dma_start(
                O[D:P, :, :], zrow.rearrange("p (h s) -> p h s", h=HPC)
            )

            with (
                tc.tile_pool(name="scps", bufs=2, space="PSUM") as scps,
                tc.tile_pool(name="avps", bufs=4, space="PSUM") as avps,
                tc.tile_pool(name="expp", bufs=3) as expp,
                tc.tile_pool(name="smal", bufs=4) as smal,
            ):
                for pr in range(2):
                    he, ho = 2 * pr, 2 * pr + 1
                    for qs in range(NQS):
                        nkb = 4 * (qs + 1)
                        qsl = slice(qs * QS, (qs + 1) * QS)
                        av_e = avps.tile([D + 1, QS], FP32, tag="av")
                        av_o = avps.tile([D + 1, QS], FP32, tag="av")
                        qt_pr = QT[:, pr, qsl]

                        def score(kb):
                            # both heads contract over all 128 partitions
                            # (dead rows of KTz are zero); shared rhs.
                            sc2 = scps.tile([P, 2, QS], FP32, tag="sc")
                            for j, h in enumerate((he, ho)):
                                nc.tensor.matmul(
                                    sc2[:, j, :],
                                    KTz[:, h, kb * P:(kb + 1) * P],
                                    qt_pr, start=True, stop=True,
                                )
                            return sc2

                        sc2 = score(0)
                        for kb in range(nkb):
                            ex2 = expp.tile([P, 2, QS], FP32R, tag="ex")
                            nc.scalar.activation(
                                ex2, sc2, EXP, scale=float(D) ** -0.5
                            )
                            if kb >= 4 * qs:  # diagonal block -> causal mask
                                m = kb - 4 * qs
                                nc.vector.tensor_mul(
                                    ex2, ex2,
                                    msk[:, m:m + 1, :].to_broadcast((P, 2, QS)),
                                )
                            if kb + 1 < nkb:
                                sc2 = score(kb + 1)
                            nc.tensor.matmul(
                                av_e, V[:, kb, he, 0:D + 1], ex2[:, 0, :],
                                start=(kb == 0), stop=(kb == nkb - 1),
                            )
                            nc.tensor.matmul(
                                av_o, V[:, kb, ho, 0:D + 1], ex2[:, 1, :],
                                start=(kb == 0), stop=(kb == nkb - 1),
                            )

                        # Denominators sit in PSUM row 64 of av_e/av_o.  HW
                        # partition_broadcast only reads partition 0: copy the
                        # rows to SBUF (base-aligned), DMA down to partition 0,
                        # then approx-reciprocal + broadcast + scale.
                        rc = smal.tile([D + 1, 2, QS], FP32, tag="rc")
                        nc.vector.tensor_copy(rc[D:D + 1, 0, :], av_e[D:D + 1, :])
                        nc.vector.tensor_copy(rc[D:D + 1, 1, :], av_o[D:D + 1, :])
                        rc0 = smal.tile([1, 2, QS], FP32, tag="rc0")
                        nc.sync.dma_start(rc0, rc[D:D + 1, :, :])
                        nc.vector.reciprocal_approx_fast(out=rc0, in_=rc0)
                        rcb = smal.tile([D, 2, QS], FP32, tag="rcb")
                        nc.gpsimd.partition_broadcast(rcb, rc0)
                        nc.vector.tensor_mul(
                            O[0:D, he, qsl], av_e[0:D, :], rcb[:, 0, :]
                        )
                        nc.vector.tensor_mul(
                            O[0:D, ho, qsl], av_o[0:D, :], rcb[:, 1, :]
                        )

            # ---- phase C: output projection (partial; host adds bias+reduce) --
            with (
                tc.tile_pool(name="pjps", bufs=4, space="PSUM") as pjps,
                tc.tile_pool(name="ysb", bufs=3) as ysb,
            ):
                for st in range(NKB):
                    for es in range(E // QS):
                        esl = slice(es * QS, (es + 1) * QS)
                        ps = pjps.tile([P, QS], FP32, tag="pj")
                        for h in range(HPC):
                            wp_t = wp_ev if h % 2 == 0 else wp_od
                            nc.tensor.matmul(
                                ps,
                                O[:, h, st * P:(st + 1) * P],
                                wp_t[:, h // 2, esl],
                                start=(h == 0), stop=(h == HPC - 1),
                            )
                        yt = ysb.tile([P, QS], FP32, tag="yt")
                        nc.vector.tensor_copy(yt, ps)
                        nc.sync.dma_start(
                            y_d[st * P:(st + 1) * P, es * QS:(es + 1) * QS], yt
                        )


def _make_masks():
    # msk[p, m, q] = 1.0 if q >= 128*m + p else 0.0
    p = np.arange(P)[:, None, None]
    m = np.arange(4)[None, :, None]
    q = np.arange(QS)[None, None, :]
    return (q >= P * m + p).astype(np.float32)


def make_in_maps(x, Wq, Wk, Wv, Wproj):
    msk = _make_masks()
    one = np.ones((P, NKB * HPC), dtype=np.float32)
    zero = np.zeros((D, HPC * S), dtype=np.float32)
    in_maps = []
    for c in range(NCORES):
        b, h0 = c // 4, HPC * (c % 4)
        hs = slice(h0, h0 + HPC)
        in_maps.append({
            "xt": np.ascontiguousarray(x[b].T),
            "wq": np.ascontiguousarray(Wq[hs].transpose(1, 0, 2).reshape(E, HPC * D)),
            "wk": np.ascontiguousarray(Wk[hs].transpose(1, 0, 2).reshape(E, HPC * D)),
            "wv": np.ascontiguousarray(Wv[hs].transpose(1, 0, 2).reshape(E, HPC * D)),
            "wp": np.ascontiguousarray(Wproj[D * h0:D * (h0 + HPC)]),
            "msk": msk,
            "one": one,
            "zero": zero,
        })
    return in_maps


def kernel(x, Wq, Wk, Wv, Wproj, bproj):
    x = np.asarray(x, dtype=np.float32)
    Wq, Wk, Wv = (np.asarray(w, dtype=np.float32) for w in (Wq, Wk, Wv))
    Wproj = np.asarray(Wproj, dtype=np.float32)
    bproj = np.asarray(bproj, dtype=np.float32)

    in_maps = make_in_maps(x, Wq, Wk, Wv, Wproj)
    nc = _build_program()
    res = bass_utils.run_bass_kernel_spmd(nc, in_maps, core_ids=list(range(NCORES)))
    parts = [r["y"] for r in res.results]
    out = np.empty((B, S, E), dtype=np.float32)
    for b in range(B):
        out[b] = parts[4 * b] + parts[4 * b + 1] + parts[4 * b + 2] + parts[4 * b + 3]
        out[b] += bproj
    return out


# revision 12
# speedup vs baseline: 1.0098x; 1.0098x over previous
"""Multi-head causal attention (B=2, S=2048, E=1024, H=16, D=64) on 8 trn2 cores.

Sharding: 32 (batch, head) pairs -> core c handles batch c//4, heads
4*(c%4) .. 4*(c%4)+4.  Each core computes QKV projections for its 4 heads,
causal flash-style attention, and a partial output projection (its heads'
rows of Wproj).  Host pre-transposes x (x^T is what every on-core matmul
wants), sums the 4 partials per batch, and adds bproj.

On-core layouts (partition dim first):
  xT   [128, 8, 2048]      x^T, e on partitions (8 per-tile DMAs)
  QT   [128, 2, 2048]      q^T pair-packed: rows 0-63 head even, 64-127 head odd
  KTz  [128, 4, 2048]      per-head k^T, zero-padded to K=128: head h occupies
                           rows 64*(h%2)..64*(h%2)+64, the other 64 rows are 0
  V    [128, 16, 4, 72]    v blocks [k, s-tile, head, d], col 64 = ones (denom)
  O    [128, 4, 2048]      per-head normalized attention output^T in rows 0-63,
                           rows 64-127 zero (K=128 projection)

Every matmul contracts over the full 128 partitions (zero-padded where the
logical contraction is 64) so the PE activity monitor keeps the array at
2.4 GHz.  Scores for a head pair share one rhs (the pair-packed Q^T tile);
one ScalarE exp covers both heads' [128, 2x512] block.  Softmax denominators
come from V's ones column; causal masking is a 0/1 broadcast-multiply on exp
tiles of diagonal blocks.  All matmuls run fp32r (TF32-like, full PE rate).
partition_broadcast on HW only reads partition 0, so denominators are moved
to partition 0 via a small SBUF->SBUF DMA before reciprocal+broadcast.
"""

import numpy as np

import concourse.bacc as bacc
import concourse.mybir as mybir
import concourse.tile as tile
from concourse import bass_utils

B, S, E, H, D = 2, 2048, 1024, 16, 64
NCORES = 8
HPC = 4            # heads per core
P = 128
QS = 512           # q slice width
NQS = S // QS      # 4
NKB = S // P       # 16 k blocks
FP32 = mybir.dt.float32
FP32R = mybir.dt.float32r
EXP = mybir.ActivationFunctionType.Exp

_CACHE = {}


def _build_program():
    if "nc" in _CACHE:
        return _CACHE["nc"]

    nc = bacc.Bacc("TRN2", target_bir_lowering=False, debug=False)
    xt_d = nc.dram_tensor("xt", (E, S), FP32, kind="ExternalInput").ap()
    wq_d = nc.dram_tensor("wq", (E, HPC * D), FP32, kind="ExternalInput").ap()
    wk_d = nc.dram_tensor("wk", (E, HPC * D), FP32, kind="ExternalInput").ap()
    wv_d = nc.dram_tensor("wv", (E, HPC * D), FP32, kind="ExternalInput").ap()
    wp_d = nc.dram_tensor("wp", (HPC * D, E), FP32, kind="ExternalInput").ap()
    msk_d = nc.dram_tensor("msk", (P, 4, QS), FP32, kind="ExternalInput").ap()
    one_d = nc.dram_tensor("one", (P, NKB * HPC), FP32, kind="ExternalInput").ap()
    zero_d = nc.dram_tensor("zero", (D, HPC * S), FP32, kind="ExternalInput").ap()
    y_d = nc.dram_tensor("y", (S, E), FP32, kind="ExternalOutput").ap()

    with tile.TileContext(nc) as tc:
        _kernel(tc, nc, xt_d, wq_d, wk_d, wv_d, wp_d, msk_d, one_d, zero_d, y_d)
    nc.compile()
    _CACHE["nc"] = nc
    return nc


def _kernel(tc, nc, xt_d, wq_d, wk_d, wv_d, wp_d, msk_d, one_d, zero_d, y_d):
    ET = E // P  # 8 e-tiles
    zrow = zero_d.bitcast(FP32R)
    with tc.tile_pool(name="persist", bufs=1) as persist:
        QT = persist.tile([P, 2, S], FP32R)
        KTz = persist.tile([P, HPC, S], FP32R)
        V = persist.tile([P, NKB, HPC, 72], FP32R)
        msk = persist.tile([P, 4, QS], FP32R)

        ones_sb = persist.tile([P, D], FP32R)
        nc.scalar.dma_start(msk, msk_d.bitcast(FP32R))
        nc.gpsimd.dma_start(
            V[:, :, :, 64],
            one_d.rearrange("p (a b) -> p a b", a=NKB).bitcast(FP32R),
        )
        nc.gpsimd.dma_start(ones_sb, one_d.bitcast(FP32R))
        # zero the dead half of each head's K^T (K=128 contraction padding)
        for h in range(HPC):
            r = slice(0, D) if h % 2 else slice(D, P)
            nc.gpsimd.dma_start(
                KTz[r, h, :], zrow[:, 0:S].rearrange("p s -> p s")
            )

        # ---- phase A: load x^T + QKV projections ----
        with (
            tc.tile_pool(name="qkvw", bufs=1) as qkvw,
            tc.tile_pool(name="qkvps", bufs=3, space="PSUM") as qkvps,
        ):
            wq_sb = qkvw.tile([P, ET, HPC * D], FP32R)
            wk_sb = qkvw.tile([P, ET, HPC * D], FP32R)
            wv_sb = qkvw.tile([P, ET, HPC * D], FP32R)
            nc.sync.dma_start(
                wq_sb, wq_d.rearrange("(et p) m -> p et m", p=P).bitcast(FP32R)
            )
            nc.scalar.dma_start(
                wk_sb, wk_d.rearrange("(et p) m -> p et m", p=P).bitcast(FP32R)
            )
            xT = qkvw.tile([P, ET, S], FP32R)
            xt_r = xt_d.rearrange("(et p) s -> p et s", p=P).bitcast(FP32R)
            dma_engines = (nc.sync, nc.gpsimd, nc.scalar)
            for et in range(ET):
                dma_engines[et % 3].dma_start(xT[:, et, :], xt_r[:, et, :])
            nc.scalar.dma_start(
                wv_sb, wv_d.rearrange("(et p) m -> p et m", p=P).bitcast(FP32R)
            )

            # Q^T / K^T: head pairs packed on partitions
            for pr in range(2):
                he, ho = 2 * pr, 2 * pr + 1
                for ss in range(NQS):
                    ssl = slice(ss * QS, (ss + 1) * QS)
                    for w_sb, is_q in ((wq_sb, True), (wk_sb, False)):
                        ps = qkvps.tile([P, QS], FP32, tag="qkps")
                        for et in range(ET):
                            nc.tensor.matmul(
                                ps,
                                w_sb[:, et, 2 * D * pr:2 * D * (pr + 1)],
                                xT[:, et, ssl],
                                start=(et == 0), stop=(et == ET - 1),
                            )
                        if is_q:
                            nc.vector.tensor_copy(QT[:, pr, ssl], ps)
                        else:
                            nc.vector.tensor_copy(KTz[0:D, he, ssl], ps[0:D, :])
                            nc.vector.tensor_copy(KTz[D:P, ho, ssl], ps[D:P, :])

            # V: [s, head*d] via xT as stationary
            for st in range(NKB):
                ps = qkvps.tile([P, HPC * D], FP32, tag="vps")
                for et in range(ET):
                    nc.tensor.matmul(
                        ps,
                        xT[:, et, st * P:(st + 1) * P],
                        wv_sb[:, et, :],
                        start=(et == 0), stop=(et == ET - 1),
                    )
                for h in range(HPC):
                    nc.vector.tensor_copy(V[:, st, h, 0:D], ps[:, D * h:D * (h + 1)])

        # ---- phase B: attention, one head-pair per unit ----
        with tc.tile_pool(name="attn", bufs=1) as attn_pool:
            O = attn_pool.tile([P, HPC, S], FP32R)
            # Per-parity wp views so the K=128 projection pairs head data in
            # O rows 0-63 with that head's wp rows on partitions 0-63; rows
            # 64-127 multiply O's zero rows (values only need to be finite).
            wp_ev = attn_pool.tile([P, 2, E], FP32R)
            wp_od = attn_pool.tile([P, 2, E], FP32R)
            wpr = wp_d.bitcast(FP32R)
            for pr in range(2):
                nc.gpsimd.dma_start(wp_ev[0:D, pr, :], wpr[P * pr:P * pr + D, :])
                nc.gpsimd.dma_start(wp_ev[D:P, pr, :], wpr[P * pr + D:P * pr + P, :])
                nc.gpsimd.dma_start(wp_od[0:D, pr, :], wpr[P * pr + D:P * pr + P, :])
                nc.gpsimd.dma_start(wp_od[D:P, pr, :], wpr[P * pr:P * pr + D, :])
            # zero rows 64-127 of O (K=128 projection padding)
            nc.gpsimd.dma_start(
                O[D:P, :, :], zrow.rearrange("p (h s) -> p h s", h=HPC)
            )

            with (
                tc.tile_pool(name="scps", bufs=2, space="PSUM") as scps,
                tc.tile_pool(name="avps", bufs=3, space="PSUM") as avps,
                tc.tile_pool(name="bcps", bufs=1, space="PSUM") as bcps,
                tc.tile_pool(name="expp", bufs=4) as expp,
                tc.tile_pool(name="smal", bufs=4) as smal,
            ):
                for pr in range(2):
                    he, ho = 2 * pr, 2 * pr + 1
                    for qs in range(NQS):
                        nkb = 4 * (qs + 1)
                        qsl = slice(qs * QS, (qs + 1) * QS)
                        av_e = avps.tile([D + 1, QS], FP32, tag="av")
                        av_o = avps.tile([D + 1, QS], FP32, tag="av")
                        qt_pr = QT[:, pr, qsl]

                        def score(kb):
                            # both heads contract over all 128 partitions
                            # (dead rows of KTz are zero); shared rhs.
                            sc2 = scps.tile([P, 2, QS], FP32, tag="sc")
                            for j, h in enumerate((he, ho)):
                                nc.tensor.matmul(
                                    sc2[:, j, :],
                                    KTz[:, h, kb * P:(kb + 1) * P],
                                    qt_pr, start=True, stop=True,
                                )
                            return sc2

                        sc2 = score(0)
                        for kb in range(nkb):
                            ex2 = expp.tile([P, 2, QS], FP32R, tag="ex")
                            nc.scalar.activation(
                                ex2, sc2, EXP, scale=float(D) ** -0.5
                            )
                            if kb >= 4 * qs:  # diagonal block -> causal mask
                                m = kb - 4 * qs
                                nc.vector.tensor_mul(
                                    ex2, ex2,
                                    msk[:, m:m + 1, :].to_broadcast((P, 2, QS)),
                                )
                            if kb + 1 < nkb:
                                sc2 = score(kb + 1)
                            nc.tensor.matmul(
                                av_e, V[:, kb, he, 0:D + 1], ex2[:, 0, :],
                                start=(kb == 0), stop=(kb == nkb - 1),
                            )
                            nc.tensor.matmul(
                                av_o, V[:, kb, ho, 0:D + 1], ex2[:, 1, :],
                                start=(kb == 0), stop=(kb == nkb - 1),
                            )

                        # Denominators sit in PSUM row 64 of av_e/av_o.
                        # Copy to SBUF (base-aligned), broadcast to rows 0-63
                        # via a K=1 ones-matmul, then approx-reciprocal into
                        # SBUF and scale.  No gpsimd / DMA in the chain.
                        rc = smal.tile([D + 1, 2, QS], FP32R, tag="rc")
                        nc.vector.tensor_copy(rc[D:D + 1, 0, :], av_e[D:D + 1, :])
                        nc.vector.tensor_copy(rc[D:D + 1, 1, :], av_o[D:D + 1, :])
                        rcb = smal.tile([D, 2, QS], FP32, tag="rcb")
                        for j in range(2):
                            bc = bcps.tile([D, QS], FP32, tag="bc")
                            nc.tensor.matmul(
                                bc, ones_sb[D:D + 1, 0:D], rc[D:D + 1, j, :],
                                start=True, stop=True,
                            )
                            nc.vector.reciprocal_approx_fast(
                                out=rcb[:, j, :], in_=bc
                            )
                        nc.vector.tensor_mul(
                            O[0:D, he, qsl], av_e[0:D, :], rcb[:, 0, :]
                        )
                        nc.vector.tensor_mul(
                            O[0:D, ho, qsl], av_o[0:D, :], rcb[:, 1, :]
                        )

            # ---- phase C: output projection (partial; host adds bias+reduce) --
            with (
                tc.tile_pool(name="pjps", bufs=4, space="PSUM") as pjps,
                tc.tile_pool(name="ysb", bufs=3) as ysb,
            ):
                for st in range(NKB):
                    for es in range(E // QS):
                        esl = slice(es * QS, (es + 1) * QS)
                        ps = pjps.tile([P, QS], FP32, tag="pj")
                        for h in range(HPC):
                            wp_t = wp_ev if h % 2 == 0 else wp_od
                            nc.tensor.matmul(
                                ps,
                                O[:, h, st * P:(st + 1) * P],
                                wp_t[:, h // 2, esl],
                                start=(h == 0), stop=(h == HPC - 1),
                            )
                        yt = ysb.tile([P, QS], FP32, tag="yt")
                        nc.vector.tensor_copy(yt, ps)
                        nc.sync.dma_start(
                            y_d[st * P:(st + 1) * P, es * QS:(es + 1) * QS], yt
                        )


def _make_masks():
    # msk[p, m, q] = 1.0 if q >= 128*m + p else 0.0
    p = np.arange(P)[:, None, None]
    m = np.arange(4)[None, :, None]
    q = np.arange(QS)[None, None, :]
    return (q >= P * m + p).astype(np.float32)


def make_in_maps(x, Wq, Wk, Wv, Wproj):
    msk = _make_masks()
    one = np.ones((P, NKB * HPC), dtype=np.float32)
    zero = np.zeros((D, HPC * S), dtype=np.float32)
    in_maps = []
    for c in range(NCORES):
        b, h0 = c // 4, HPC * (c % 4)
        hs = slice(h0, h0 + HPC)
        in_maps.append({
            "xt": np.ascontiguousarray(x[b].T),
            "wq": np.ascontiguousarray(Wq[hs].transpose(1, 0, 2).reshape(E, HPC * D)),
            "wk": np.ascontiguousarray(Wk[hs].transpose(1, 0, 2).reshape(E, HPC * D)),
            "wv": np.ascontiguousarray(Wv[hs].transpose(1, 0, 2).reshape(E, HPC * D)),
            "wp": np.ascontiguousarray(Wproj[D * h0:D * (h0 + HPC)]),
            "msk": msk,
            "one": one,
            "zero": zero,
        })
    return in_maps


def kernel(x, Wq, Wk, Wv, Wproj, bproj):
    x = np.asarray(x, dtype=np.float32)
    Wq, Wk, Wv = (np.asarray(w, dtype=np.float32) for w in (Wq, Wk, Wv))
    Wproj = np.asarray(Wproj, dtype=np.float32)
    bproj = np.asarray(bproj, dtype=np.float32)

    in_maps = make_in_maps(x, Wq, Wk, Wv, Wproj)
    nc = _build_program()
    res = bass_utils.run_bass_kernel_spmd(nc, in_maps, core_ids=list(range(NCORES)))
    parts = [r["y"] for r in res.results]
    out = np.empty((B, S, E), dtype=np.float32)
    for b in range(B):
        out[b] = parts[4 * b] + parts[4 * b + 1] + parts[4 * b + 2] + parts[4 * b + 3]
        out[b] += bproj
    return out
